# revision 30
# baseline (speedup 1.0000x reference)
"""Multi-head attention (B=4, S=2048, D=512, H=8) on 8 TRN2 NeuronCores.

Sharding: core c handles batch b = c//2 and head-group g = c%2 (heads
4g..4g+3, a 256-dim slice of the model). Attention is independent per
(batch, head); the out-projection contracts over all 512 dims, so each core
computes a partial out = attn_g @ W_out[g] and the host sums the two
partials per batch. x^T inputs are marshalled on the host; projection
weights are sliced per head-group (no K/V duplication).

Per-core dataflow:
  1. qT/kT = (x W)^T in [256, 2048] dims-major fp32r layout; v in natural
     [2048, 256] + a ones column per (key-block, head) -> v_aug fp16.
  2. Per slot (query-half, head): scores^T st[128k, 1024q] fp32r in PSUM;
     softmax numerators pT[128k, 1024q] fp16 produced by a mixed engine
     schedule: ACT exact exp / DVE+Pool Schraudolph int16-bitcast exp
     (single or phase-averaged pair for 4x better accuracy). Pool cannot
     read PSUM, so its inputs are DMA-staged to SBUF.
  3. PV in natural orientation: pv[128q, 65] += pT^T @ v_aug per
     (q-block, key-block) - the 65-wide free dim (64 dims + denominator
     column) makes each accumulation step cost 65 PE cycles instead of
     the 1024 a dims-major PV would pay. Normalization (1/denominator)
     is fused into the PSUM->SBUF copy via a per-partition scalar.
  4. attn[q,dh] tiles are transposed back to dims-major via PE-transpose
     against a host-provided identity, then out = attnT^T @ W_out (fp16)
     streams per 128-query chunk.
"""

import numpy as np

import concourse.bass as bass
from concourse import bacc
import concourse.mybir as mybir
import concourse.tile as tile
from concourse.bass_utils import run_bass_kernel_spmd

B, S, D, H = 4, 2048, 512, 8
DH = 64
P = 128
NCORES = 8
HG = H // 2          # 4 heads per core
DG = HG * DH         # 256 dims per core
NKB = S // P         # 16 key blocks
VW = DH + 1          # 65
F32 = mybir.dt.float32
F32R = mybir.dt.float32r
F16 = mybir.dt.float16
I16 = mybir.dt.int16
EXP = mybir.ActivationFunctionType.Exp
MUL = mybir.AluOpType.mult
ADD = mybir.AluOpType.add
SCALE = 1.0 / np.sqrt(DH)            # 0.125
LOG2E = 1.4426950408889634
A16 = SCALE * LOG2E * 1024.0         # fp16-bits Schraudolph slope
B_SS = 15.0 * 1024.0 - 59.0          # single Schraudolph bias (mean-1)
# Equal-weight phase pair: exp(x) ~ bits(x*A+B_PH1).f16 + bits(x*A+B_PH2).f16
# (CV 0.53%, zero mean bias; the 0.5 weights are folded into the biases so
# the combine is a single fp16 add, which Pool supports.)
B_PH1 = 15.0 * 1024.0 - 1024.0 - 332.0
B_OFF = 504.0  # S2 = S1 + B_OFF in int16 bits (half-period phase shift)

# Per-slot exp engine schedule over the 16 key blocks:
#  A  = exact exp on ACT
#  PH = phase-averaged Schraudolph pair on DVE; the two halves are never
#       added explicitly - PV accumulates both (PV is linear in p), so the
#       fp16 "combine" costs a few extra 65-cycle PE matmuls instead of a
#       slow Pool tensor op.
SCHED = ["A", "D", "A", "D", "A", "A", "PH", "A",
         "A", "D", "A", "D", "A", "PH", "A", "A"]
PH_BLOCKS = [k for k, s in enumerate(SCHED) if s == "PH"]
PH_IDX = {k: i for i, k in enumerate(PH_BLOCKS)}


def _build_mha(tc, out_d, xqT_d, xkT_d, xvT_d, wq_d, wk_d, wv_d, wo_d, ident_d):
    nc = tc.nc

    dma_rr = [0]

    def dma(dst, src):
        eng = nc.sync if dma_rr[0] % 2 == 0 else nc.scalar
        dma_rr[0] += 1
        eng.dma_start(dst, src)

    copy_rr = [0]

    def pcopy(dst, src):
        if copy_rr[0] % 2 == 0:
            nc.scalar.copy(dst, src)
        else:
            nc.vector.tensor_copy(dst, src)
        copy_rr[0] += 1

    with (
        tc.tile_pool(name="consts", bufs=1) as cpool,
        tc.tile_pool(name="big", bufs=1) as bpool,
        tc.tile_pool(name="work", bufs=2) as wpool,
    ):
        wq_sb = cpool.tile([P, 4, DG], F16)
        wk_sb = cpool.tile([P, 4, DG], F16)
        wv_sb = cpool.tile([P, 4, DG], F16)
        wo_sb = cpool.tile([P, 2, D], F16)
        ident = cpool.tile([P, P], F16)
        xq_sb = bpool.tile([P, 4, S], F16)   # [din%128, din//128, s]
        xk_sb = bpool.tile([P, 4, S], F16)
        xv_sb = bpool.tile([P, 4, S], F16)
        qT = bpool.tile([P, 2, S], F16)      # [dim%128, dim//128, q]
        kT = bpool.tile([P, 2, S], F16)
        v_aug = bpool.tile([P, NKB, HG, VW], F16)  # [k%128, k//128, h, dh|1]
        attnT = bpool.tile([P, 2, S], F16)   # [dim%128, dim//128, q]
        scratch = cpool.tile([P, 512], F16)
        warm = cpool.tile([P, 1], F16)

        # gpsimd lane: scratch memset + ones-fill first, then SWDGE DMAs
        # (which bypass the serialized HWDGE transfer resource).
        nc.gpsimd.memset(scratch, 0)
        nc.gpsimd.tensor_scalar(
            out=v_aug.rearrange("p a b e -> p (a b) e")[:, :, DH],
            in0=scratch[:, 0:1].broadcast_to([P, NKB * HG]),
            scalar1=0.0,
            scalar2=1.0,
            op0=MUL,
            op1=ADD,
        )

        # Pull the ACT exp-table load to t=0 (input values are arbitrary).
        nc.scalar.activation(warm, scratch[:, 0:1], EXP)

        # Transfers share one serialized DMA resource served round-robin
        # across queue heads; spread strips so q/k arrive earliest.
        nc.sync.dma_start(wq_sb, wq_d.rearrange("(c p) n -> p c n", p=P))
        nc.scalar.dma_start(wk_sb, wk_d.rearrange("(c p) n -> p c n", p=P))
        nc.gpsimd.dma_start(wv_sb, wv_d.rearrange("(c p) n -> p c n", p=P))
        for c in range(2):
            nc.sync.dma_start(xq_sb[:, c, :], xqT_d[c * P : (c + 1) * P, :])
            nc.gpsimd.dma_start(
                xq_sb[:, 2 + c, :], xqT_d[(2 + c) * P : (3 + c) * P, :]
            )
        for c in range(2):
            nc.gpsimd.dma_start(xk_sb[:, c, :], xkT_d[c * P : (c + 1) * P, :])
            nc.scalar.dma_start(
                xk_sb[:, 2 + c, :], xkT_d[(2 + c) * P : (3 + c) * P, :]
            )
        for c in range(4):
            nc.gpsimd.dma_start(xv_sb[:, c, :], xvT_d[c * P : (c + 1) * P, :])
        nc.scalar.dma_start(wo_sb, wo_d.rearrange("(c p) n -> p c n", p=P))
        nc.sync.dma_start(ident, ident_d)

        # ---------------- attention pools (aux outermost, then st) --------
        # aux carries projection psums, then PV psums, then the final PV(7);
        # one uniform [P, 512] tag so its 2 banks are reused across phases.
        aux_cm = tc.tile_pool(name="ps_aux", bufs=2, space="PSUM")
        aux = aux_cm.__enter__()
        st_cm = tc.tile_pool(name="ps_st", bufs=3, space="PSUM")
        st_pool = st_cm.__enter__()

        # PE pre-ramp into a throwaway st-pool tile.
        ramp = st_pool.tile([P, 1024], F32, tag="st", name="ramp")
        for _ in range(13):
            nc.tensor.matmul(
                ramp[:, 0:512], scratch[:, 0:P], scratch, start=True, stop=True
            )

        def project_col(x_sb, w_sb, dst, mc, col):
            pp = aux.tile([P, 512], F32, tag="qk", name=f"pp{mc}{col}")
            for c in range(4):
                nc.tensor.matmul(
                    pp,
                    w_sb[:, c, mc * P : (mc + 1) * P],
                    x_sb[:, c, col * 512 : (col + 1) * 512],
                    start=(c == 0),
                    stop=(c == 3),
                )
            pcopy(dst[:, mc, col * 512 : (col + 1) * 512], pp)

        # Head: only the mc0 halves of Q/K (all slot-0/1 scores need just
        # dims 0..127); mc1 halves + V weave into slots 0/1 as PE filler.
        for col in range(4):
            project_col(xq_sb, wq_sb, qT, 0, col)
        for col in range(4):
            project_col(xk_sb, wk_sb, kT, 0, col)

        def emit_v_proj(kblk):
            # V projection psum shares the aux qk buffers (1 bank each).
            vt = aux.tile([P, 512], F32, tag="qk", name=f"vp{kblk}")
            vv = vt[:, 0:260].rearrange("p (a b) -> p a b", b=VW)
            for c in range(4):
                nc.tensor.matmul(
                    vv[:, :, 0:DH],
                    xv_sb[:, c, kblk * P : (kblk + 1) * P],
                    wv_sb[:, c, :],
                    start=(c == 0),
                    stop=(c == 3),
                )
            pcopy(v_aug[:, kblk, :, 0:DH], vv[:, :, 0:DH])

        # ---------------- attention slots ----------------
        slots = [(qh, h) for qh in (0, 1) for h in range(4)]

        def alloc_pv(name):
            t = aux.tile([P, 512], F32, tag="qk", name=name)
            return t[:, 0:260].rearrange("p (a b) -> p a b", b=VW)
        aT_pool = [None]
        opj_pool = [None]

        pT_tiles = {}
        pT2_tiles = {}
        attn_tiles = {}

        def emit_score_block(i, kblk):
            qh, h = slots[i]
            po, mc = DH * (h % 2), h // 2
            pT = pT_tiles[i]
            st = st_pool.tile([P, 1024], F32, tag="st")
            qoff = qh * 1024
            for qc in range(2):
                nc.tensor.matmul(
                    st[:, qc * 512 : (qc + 1) * 512],
                    kT[po : po + DH, mc, kblk * P : (kblk + 1) * P],
                    qT[po : po + DH, mc, qoff + qc * 512 : qoff + (qc + 1) * 512],
                    start=True,
                    stop=True,
                )
            kind = SCHED[kblk]
            dst16 = pT[:, kblk, :]
            if kind == "A":
                nc.scalar.activation(dst16, st, EXP, scale=float(SCALE))
            elif kind == "D":
                nc.vector.tensor_scalar(
                    out=dst16.bitcast(I16), in0=st,
                    scalar1=A16, scalar2=B_SS, op0=MUL, op1=ADD,
                )
            else:  # PH: half 1 into pT, half 2 (bits + B_OFF) into pT2
                nc.vector.tensor_scalar(
                    out=dst16.bitcast(I16), in0=st,
                    scalar1=A16, scalar2=B_PH1, op0=MUL, op1=ADD,
                )
                pT2 = pT2_tiles[i]
                nc.gpsimd.tensor_scalar(
                    out=pT2.bitcast(I16)[:, PH_IDX[kblk], :],
                    in0=dst16.bitcast(I16),
                    scalar1=float(B_OFF),
                    scalar2=None,
                    op0=ADD,
                )

        def emit_pv_piece(j, pv, half, jj):
            qh, h = slots[j]
            pT = pT_tiles[j]
            pT2 = pT2_tiles[j]
            qblk = half * 4 + jj
            for kblk in range(NKB):
                nc.tensor.matmul(
                    pv[:, jj, :],
                    pT[:, kblk, qblk * P : (qblk + 1) * P],
                    v_aug[:, kblk, h, :],
                    start=(kblk == 0),
                    stop=False,
                )
            for pi, kblk in enumerate(PH_BLOCKS):
                nc.tensor.matmul(
                    pv[:, jj, :],
                    pT2[:, pi, qblk * P : (qblk + 1) * P],
                    v_aug[:, kblk, h, :],
                    start=False,
                    stop=(pi == len(PH_BLOCKS) - 1),
                )

        def emit_norm_half(j, half, pv):
            if half == 0:
                attn_tiles[j] = wpool.tile([P, 8, DH], F16, tag="attn", bufs=8, name=f"attn{j}")
            attn = attn_tiles[j]
            recip = wpool.tile([P, 4], F32, tag="recip", bufs=2)
            nc.vector.reciprocal(recip, pv[:, :, DH])
            for jj in range(4):
                nc.vector.tensor_scalar(
                    out=attn[:, half * 4 + jj, :],
                    in0=pv[:, jj, 0:DH],
                    scalar1=recip[:, jj : jj + 1],
                    scalar2=None,
                    op0=MUL,
                )

        def emit_transpose(j):
            qh, h = slots[j]
            po, mc = DH * (h % 2), h // 2
            attn = attn_tiles.pop(j)
            aT = aT_pool[0].tile([DH, 1024], F16, tag="aT")
            for qblk in range(8):
                nc.tensor.transpose(
                    aT[:, qblk * P : (qblk + 1) * P], attn[:, qblk, :], ident
                )
            nc.vector.tensor_copy(
                attnT[po : po + DH, mc, qh * 1024 : (qh + 1) * 1024], aT
            )

        def emit_opj(qch):
            op = opj_pool[0].tile([P, D], F32, tag="opj")
            for c in range(2):
                nc.tensor.matmul(
                    op,
                    attnT[:, c, qch * P : (qch + 1) * P],
                    wo_sb[:, c, :],
                    start=(c == 0),
                    stop=(c == 1),
                )
            ob = wpool.tile([P, D], F16, tag="ob", bufs=4)
            if qch % 2 == 0:
                nc.scalar.copy(ob, op)
            else:
                nc.vector.tensor_copy(ob, op)
            oq = (nc.gpsimd, nc.sync, nc.gpsimd, nc.scalar)[qch % 4]
            oq.dma_start(out_d[qch * P : (qch + 1) * P, :], ob)

        for i in range(8):
            pv_a = pv_b = None
            if True:
                pT_tiles[i] = wpool.tile([P, NKB, 1024], F16, tag="pT", bufs=2, name=f"pT{i}")
                pT2_tiles[i] = wpool.tile(
                    [P, len(PH_BLOCKS), 1024], F16, tag="pT2", bufs=2, name=f"pT2_{i}"
                )
                for kblk in range(NKB):
                    emit_score_block(i, kblk)
                    if i == 0:
                        if kblk < 4:
                            project_col(xq_sb, wq_sb, qT, 1, kblk)
                        elif kblk < 8:
                            project_col(xk_sb, wk_sb, kT, 1, kblk - 4)
                        else:
                            emit_v_proj(kblk - 8)
                    if i >= 1 and kblk % 2 == 1:
                        # weave one PV(i-1) q-block between score pairs
                        jj = (kblk - 1) // 2
                        if jj == 0:
                            pv_a = alloc_pv(f"pvA{i}")
                        if jj == 4:
                            pv_b = alloc_pv(f"pvB{i}")
                        if jj < 4:
                            emit_pv_piece(i - 1, pv_a, 0, jj)
                        else:
                            emit_pv_piece(i - 1, pv_b, 1, jj - 4)
                        if jj == 3:
                            emit_norm_half(i - 1, 0, pv_a)
                if i == 0:
                    for kblk in range(8, NKB):
                        emit_v_proj(kblk)
                if pv_b is not None:
                    emit_norm_half(i - 1, 1, pv_b)
                    del pT_tiles[i - 1]
                    del pT2_tiles[i - 1]

        # ---------------- tail ----------------
        # st closes now; transposes + qh0 out-projection overlap PV(7).
        st_cm.__exit__(None, None, None)
        aT_cm = tc.tile_pool(name="ps_aT", bufs=2, space="PSUM")
        opj_cm = tc.tile_pool(name="ps_opj", bufs=4, space="PSUM")
        aT_pool[0] = aT_cm.__enter__()
        opj_pool[0] = opj_cm.__enter__()
        for j in range(4):
            emit_transpose(j)
        for qch in range(4):
            emit_opj(qch)
        emit_transpose(4)
        for qch in range(4, 8):
            emit_opj(qch)
        emit_transpose(5)
        pv_a = alloc_pv("pvA8")
        for jj in range(4):
            emit_pv_piece(7, pv_a, 0, jj)
        emit_transpose(6)
        emit_norm_half(7, 0, pv_a)
        pv_b = alloc_pv("pvB8")
        for jj in range(4):
            emit_pv_piece(7, pv_b, 1, jj)
        emit_norm_half(7, 1, pv_b)
        del pT_tiles[7]
        del pT2_tiles[7]
        emit_transpose(7)
        for qch in range(8, 16):
            emit_opj(qch)
        opj_cm.__exit__(None, None, None)
        aT_cm.__exit__(None, None, None)
        aux_cm.__exit__(None, None, None)


_CACHED_NC = None


def _get_nc():
    global _CACHED_NC
    if _CACHED_NC is not None:
        return _CACHED_NC
    nc = bacc.Bacc("TRN2", target_bir_lowering=False, debug=False)
    xqT = nc.dram_tensor("xqT", [D, S], F16, kind="ExternalInput").ap()
    xkT = nc.dram_tensor("xkT", [D, S], F16, kind="ExternalInput").ap()
    xvT = nc.dram_tensor("xvT", [D, S], F16, kind="ExternalInput").ap()
    wq = nc.dram_tensor("wq", [D, DG], F16, kind="ExternalInput").ap()
    wk = nc.dram_tensor("wk", [D, DG], F16, kind="ExternalInput").ap()
    wv = nc.dram_tensor("wv", [D, DG], F16, kind="ExternalInput").ap()
    wo = nc.dram_tensor("wo", [DG, D], F16, kind="ExternalInput").ap()
    ident = nc.dram_tensor("ident", [P, P], F16, kind="ExternalInput").ap()
    out = nc.dram_tensor("out", [S, D], F16, kind="ExternalOutput").ap()
    with tile.TileContext(nc) as tc:
        _build_mha(tc, out, xqT, xkT, xvT, wq, wk, wv, wo, ident)
    nc.compile()
    _CACHED_NC = nc
    return nc


def _run(in_query, in_key, in_value, W_q, W_k, W_v, W_out, **run_kwargs):
    f = lambda a: np.ascontiguousarray(np.asarray(a), dtype=np.float32)
    in_query, in_key, in_value = f(in_query), f(in_key), f(in_value)
    W_q, W_k, W_v, W_out = f(W_q), f(W_k), f(W_v), f(W_out)
    g16 = lambda a: np.ascontiguousarray(a, dtype=np.float16)
    xqT = [g16(in_query[b].T) for b in range(B)]
    xkT = [g16(in_key[b].T) for b in range(B)]
    xvT = [g16(in_value[b].T) for b in range(B)]
    ident = np.eye(P, dtype=np.float16)
    in_maps = []
    for c in range(NCORES):
        b, g = c // 2, c % 2
        in_maps.append(
            {
                "xqT": xqT[b],
                "xkT": xkT[b],
                "xvT": xvT[b],
                "wq": g16(W_q[:, g * DG : (g + 1) * DG]),
                "wk": g16(W_k[:, g * DG : (g + 1) * DG]),
                "wv": g16(W_v[:, g * DG : (g + 1) * DG]),
                "wo": np.ascontiguousarray(
                    W_out[g * DG : (g + 1) * DG, :].astype(np.float16)
                ),
                "ident": ident,
            }
        )
    res = run_bass_kernel_spmd(_get_nc(), in_maps, list(range(NCORES)), **run_kwargs)
    out = np.empty((B, S, D), np.float32)
    for b in range(B):
        out[b] = res.results[2 * b]["out"].astype(np.float32) + res.results[
            2 * b + 1
        ]["out"].astype(np.float32)
    return out, res


def kernel(in_query, in_key, in_value, W_q, W_k, W_v, W_out):
    out, _ = _run(in_query, in_key, in_value, W_q, W_k, W_v, W_out)
    return out


# revision 31
# speedup vs baseline: 1.0001x; 1.0001x over previous
"""Multi-head attention (B=4, S=2048, D=512, H=8) on 8 TRN2 NeuronCores.

Sharding: core c handles batch b = c//2 and head-group g = c%2 (heads
4g..4g+3, a 256-dim slice of the model). Attention is independent per
(batch, head); the out-projection contracts over all 512 dims, so each core
computes a partial out = attn_g @ W_out[g] and the host sums the two
partials per batch. x^T inputs are marshalled on the host; projection
weights are sliced per head-group (no K/V duplication).

Per-core dataflow:
  1. qT/kT = (x W)^T in [256, 2048] dims-major fp32r layout; v in natural
     [2048, 256] + a ones column per (key-block, head) -> v_aug fp16.
  2. Per slot (query-half, head): scores^T st[128k, 1024q] fp32r in PSUM;
     softmax numerators pT[128k, 1024q] fp16 produced by a mixed engine
     schedule: ACT exact exp / DVE+Pool Schraudolph int16-bitcast exp
     (single or phase-averaged pair for 4x better accuracy). Pool cannot
     read PSUM, so its inputs are DMA-staged to SBUF.
  3. PV in natural orientation: pv[128q, 65] += pT^T @ v_aug per
     (q-block, key-block) - the 65-wide free dim (64 dims + denominator
     column) makes each accumulation step cost 65 PE cycles instead of
     the 1024 a dims-major PV would pay. Normalization (1/denominator)
     is fused into the PSUM->SBUF copy via a per-partition scalar.
  4. attn[q,dh] tiles are transposed back to dims-major via PE-transpose
     against a host-provided identity, then out = attnT^T @ W_out (fp16)
     streams per 128-query chunk.
"""

import numpy as np

import concourse.bass as bass
from concourse import bacc
import concourse.mybir as mybir
import concourse.tile as tile
from concourse.bass_utils import run_bass_kernel_spmd

B, S, D, H = 4, 2048, 512, 8
DH = 64
P = 128
NCORES = 8
HG = H // 2          # 4 heads per core
DG = HG * DH         # 256 dims per core
NKB = S // P         # 16 key blocks
VW = DH + 1          # 65
F32 = mybir.dt.float32
F32R = mybir.dt.float32r
F16 = mybir.dt.float16
I16 = mybir.dt.int16
EXP = mybir.ActivationFunctionType.Exp
MUL = mybir.AluOpType.mult
ADD = mybir.AluOpType.add
SCALE = 1.0 / np.sqrt(DH)            # 0.125
LOG2E = 1.4426950408889634
A16 = SCALE * LOG2E * 1024.0         # fp16-bits Schraudolph slope
B_SS = 15.0 * 1024.0 - 59.0          # single Schraudolph bias (mean-1)
# Equal-weight phase pair: exp(x) ~ bits(x*A+B_PH1).f16 + bits(x*A+B_PH2).f16
# (CV 0.53%, zero mean bias; the 0.5 weights are folded into the biases so
# the combine is a single fp16 add, which Pool supports.)
B_PH1 = 15.0 * 1024.0 - 1024.0 - 332.0
B_OFF = 504.0  # S2 = S1 + B_OFF in int16 bits (half-period phase shift)

# Per-slot exp engine schedule over the 16 key blocks:
#  A  = exact exp on ACT
#  PH = phase-averaged Schraudolph pair on DVE; the two halves are never
#       added explicitly - PV accumulates both (PV is linear in p), so the
#       fp16 "combine" costs a few extra 65-cycle PE matmuls instead of a
#       slow Pool tensor op.
SCHED = ["A", "PH", "A", "D", "A", "A", "PH", "A",
         "A", "D", "A", "D", "A", "PH", "A", "A"]
PH_BLOCKS = [k for k, s in enumerate(SCHED) if s == "PH"]
PH_IDX = {k: i for i, k in enumerate(PH_BLOCKS)}


def _build_mha(tc, out_d, xqT_d, xkT_d, xvT_d, wq_d, wk_d, wv_d, wo_d, ident_d):
    nc = tc.nc

    dma_rr = [0]

    def dma(dst, src):
        eng = nc.sync if dma_rr[0] % 2 == 0 else nc.scalar
        dma_rr[0] += 1
        eng.dma_start(dst, src)

    copy_rr = [0]

    def pcopy(dst, src):
        if copy_rr[0] % 2 == 0:
            nc.scalar.copy(dst, src)
        else:
            nc.vector.tensor_copy(dst, src)
        copy_rr[0] += 1

    with (
        tc.tile_pool(name="consts", bufs=1) as cpool,
        tc.tile_pool(name="big", bufs=1) as bpool,
        tc.tile_pool(name="work", bufs=2) as wpool,
    ):
        wq_sb = cpool.tile([P, 4, DG], F16)
        wk_sb = cpool.tile([P, 4, DG], F16)
        wv_sb = cpool.tile([P, 4, DG], F16)
        wo_sb = cpool.tile([P, 2, D], F16)
        ident = cpool.tile([P, P], F16)
        xq_sb = bpool.tile([P, 4, S], F16)   # [din%128, din//128, s]
        xk_sb = bpool.tile([P, 4, S], F16)
        xv_sb = bpool.tile([P, 4, S], F16)
        qT = bpool.tile([P, 2, S], F16)      # [dim%128, dim//128, q]
        kT = bpool.tile([P, 2, S], F16)
        v_aug = bpool.tile([P, NKB, HG, VW], F16)  # [k%128, k//128, h, dh|1]
        attnT = bpool.tile([P, 2, S], F16)   # [dim%128, dim//128, q]
        scratch = cpool.tile([P, 512], F16)
        warm = cpool.tile([P, 1], F16)

        # gpsimd lane: scratch memset + ones-fill first, then SWDGE DMAs
        # (which bypass the serialized HWDGE transfer resource).
        nc.gpsimd.memset(scratch, 0)
        nc.gpsimd.tensor_scalar(
            out=v_aug.rearrange("p a b e -> p (a b) e")[:, :, DH],
            in0=scratch[:, 0:1].broadcast_to([P, NKB * HG]),
            scalar1=0.0,
            scalar2=1.0,
            op0=MUL,
            op1=ADD,
        )

        # Pull the ACT exp-table load to t=0 (input values are arbitrary).
        nc.scalar.activation(warm, scratch[:, 0:1], EXP)

        # Transfers share one serialized DMA resource served round-robin
        # across queue heads; spread strips so q/k arrive earliest.
        nc.sync.dma_start(wq_sb, wq_d.rearrange("(c p) n -> p c n", p=P))
        nc.scalar.dma_start(wk_sb, wk_d.rearrange("(c p) n -> p c n", p=P))
        nc.gpsimd.dma_start(wv_sb, wv_d.rearrange("(c p) n -> p c n", p=P))
        for c in range(2):
            nc.sync.dma_start(xq_sb[:, c, :], xqT_d[c * P : (c + 1) * P, :])
            nc.gpsimd.dma_start(
                xq_sb[:, 2 + c, :], xqT_d[(2 + c) * P : (3 + c) * P, :]
            )
        for c in range(2):
            nc.gpsimd.dma_start(xk_sb[:, c, :], xkT_d[c * P : (c + 1) * P, :])
            nc.scalar.dma_start(
                xk_sb[:, 2 + c, :], xkT_d[(2 + c) * P : (3 + c) * P, :]
            )
        for c in range(4):
            nc.gpsimd.dma_start(xv_sb[:, c, :], xvT_d[c * P : (c + 1) * P, :])
        nc.scalar.dma_start(wo_sb, wo_d.rearrange("(c p) n -> p c n", p=P))
        nc.sync.dma_start(ident, ident_d)

        # ---------------- attention pools (aux outermost, then st) --------
        # aux carries projection psums, then PV psums, then the final PV(7);
        # one uniform [P, 512] tag so its 2 banks are reused across phases.
        aux_cm = tc.tile_pool(name="ps_aux", bufs=2, space="PSUM")
        aux = aux_cm.__enter__()
        st_cm = tc.tile_pool(name="ps_st", bufs=3, space="PSUM")
        st_pool = st_cm.__enter__()

        # PE pre-ramp into a throwaway st-pool tile.
        ramp = st_pool.tile([P, 1024], F32, tag="st", name="ramp")
        for _ in range(13):
            nc.tensor.matmul(
                ramp[:, 0:512], scratch[:, 0:P], scratch, start=True, stop=True
            )

        def project_col(x_sb, w_sb, dst, mc, col):
            pp = aux.tile([P, 512], F32, tag="qk", name=f"pp{mc}{col}")
            for c in range(4):
                nc.tensor.matmul(
                    pp,
                    w_sb[:, c, mc * P : (mc + 1) * P],
                    x_sb[:, c, col * 512 : (col + 1) * 512],
                    start=(c == 0),
                    stop=(c == 3),
                )
            pcopy(dst[:, mc, col * 512 : (col + 1) * 512], pp)

        # Head: only the mc0 halves of Q/K (all slot-0/1 scores need just
        # dims 0..127); mc1 halves + V weave into slots 0/1 as PE filler.
        for col in range(4):
            project_col(xq_sb, wq_sb, qT, 0, col)
        for col in range(4):
            project_col(xk_sb, wk_sb, kT, 0, col)

        def emit_v_proj(kblk):
            # V projection psum shares the aux qk buffers (1 bank each).
            vt = aux.tile([P, 512], F32, tag="qk", name=f"vp{kblk}")
            vv = vt[:, 0:260].rearrange("p (a b) -> p a b", b=VW)
            for c in range(4):
                nc.tensor.matmul(
                    vv[:, :, 0:DH],
                    xv_sb[:, c, kblk * P : (kblk + 1) * P],
                    wv_sb[:, c, :],
                    start=(c == 0),
                    stop=(c == 3),
                )
            pcopy(v_aug[:, kblk, :, 0:DH], vv[:, :, 0:DH])

        # ---------------- attention slots ----------------
        slots = [(qh, h) for qh in (0, 1) for h in range(4)]

        def alloc_pv(name):
            t = aux.tile([P, 512], F32, tag="qk", name=name)
            return t[:, 0:260].rearrange("p (a b) -> p a b", b=VW)
        aT_pool = [None]
        opj_pool = [None]

        pT_tiles = {}
        pT2_tiles = {}
        attn_tiles = {}

        def emit_score_block(i, kblk):
            qh, h = slots[i]
            po, mc = DH * (h % 2), h // 2
            pT = pT_tiles[i]
            st = st_pool.tile([P, 1024], F32, tag="st")
            qoff = qh * 1024
            for qc in range(2):
                nc.tensor.matmul(
                    st[:, qc * 512 : (qc + 1) * 512],
                    kT[po : po + DH, mc, kblk * P : (kblk + 1) * P],
                    qT[po : po + DH, mc, qoff + qc * 512 : qoff + (qc + 1) * 512],
                    start=True,
                    stop=True,
                )
            kind = SCHED[kblk]
            dst16 = pT[:, kblk, :]
            if kind == "A":
                nc.scalar.activation(dst16, st, EXP, scale=float(SCALE))
            elif kind == "D":
                nc.vector.tensor_scalar(
                    out=dst16.bitcast(I16), in0=st,
                    scalar1=A16, scalar2=B_SS, op0=MUL, op1=ADD,
                )
            else:  # PH: half 1 into pT, half 2 (bits + B_OFF) into pT2
                nc.vector.tensor_scalar(
                    out=dst16.bitcast(I16), in0=st,
                    scalar1=A16, scalar2=B_PH1, op0=MUL, op1=ADD,
                )
                pT2 = pT2_tiles[i]
                nc.gpsimd.tensor_scalar(
                    out=pT2.bitcast(I16)[:, PH_IDX[kblk], :],
                    in0=dst16.bitcast(I16),
                    scalar1=float(B_OFF),
                    scalar2=None,
                    op0=ADD,
                )

        def emit_pv_piece(j, pv, half, jj):
            qh, h = slots[j]
            pT = pT_tiles[j]
            pT2 = pT2_tiles[j]
            qblk = half * 4 + jj
            for kblk in range(NKB):
                nc.tensor.matmul(
                    pv[:, jj, :],
                    pT[:, kblk, qblk * P : (qblk + 1) * P],
                    v_aug[:, kblk, h, :],
                    start=(kblk == 0),
                    stop=False,
                )
            for pi, kblk in enumerate(PH_BLOCKS):
                nc.tensor.matmul(
                    pv[:, jj, :],
                    pT2[:, pi, qblk * P : (qblk + 1) * P],
                    v_aug[:, kblk, h, :],
                    start=False,
                    stop=(pi == len(PH_BLOCKS) - 1),
                )

        def emit_norm_half(j, half, pv):
            if half == 0:
                attn_tiles[j] = wpool.tile([P, 8, DH], F16, tag="attn", bufs=8, name=f"attn{j}")
            attn = attn_tiles[j]
            recip = wpool.tile([P, 4], F32, tag="recip", bufs=2)
            nc.vector.reciprocal(recip, pv[:, :, DH])
            for jj in range(4):
                nc.vector.tensor_scalar(
                    out=attn[:, half * 4 + jj, :],
                    in0=pv[:, jj, 0:DH],
                    scalar1=recip[:, jj : jj + 1],
                    scalar2=None,
                    op0=MUL,
                )

        def emit_transpose(j):
            qh, h = slots[j]
            po, mc = DH * (h % 2), h // 2
            attn = attn_tiles.pop(j)
            aT = aT_pool[0].tile([DH, 1024], F16, tag="aT")
            for qblk in range(8):
                nc.tensor.transpose(
                    aT[:, qblk * P : (qblk + 1) * P], attn[:, qblk, :], ident
                )
            nc.vector.tensor_copy(
                attnT[po : po + DH, mc, qh * 1024 : (qh + 1) * 1024], aT
            )

        def emit_opj(qch):
            op = opj_pool[0].tile([P, D], F32, tag="opj")
            for c in range(2):
                nc.tensor.matmul(
                    op,
                    attnT[:, c, qch * P : (qch + 1) * P],
                    wo_sb[:, c, :],
                    start=(c == 0),
                    stop=(c == 1),
                )
            ob = wpool.tile([P, D], F16, tag="ob", bufs=4)
            if qch % 2 == 0:
                nc.scalar.copy(ob, op)
            else:
                nc.vector.tensor_copy(ob, op)
            oq = (nc.gpsimd, nc.sync, nc.gpsimd, nc.scalar)[qch % 4]
            oq.dma_start(out_d[qch * P : (qch + 1) * P, :], ob)

        for i in range(8):
            pv_a = pv_b = None
            if True:
                pT_tiles[i] = wpool.tile([P, NKB, 1024], F16, tag="pT", bufs=2, name=f"pT{i}")
                pT2_tiles[i] = wpool.tile(
                    [P, len(PH_BLOCKS), 1024], F16, tag="pT2", bufs=2, name=f"pT2_{i}"
                )
                for kblk in range(NKB):
                    emit_score_block(i, kblk)
                    if i == 0:
                        if kblk < 4:
                            project_col(xq_sb, wq_sb, qT, 1, kblk)
                        elif kblk < 8:
                            project_col(xk_sb, wk_sb, kT, 1, kblk - 4)
                        else:
                            emit_v_proj(kblk - 8)
                    if i >= 1 and kblk % 2 == 1:
                        # weave one PV(i-1) q-block between score pairs
                        jj = (kblk - 1) // 2
                        if jj == 0:
                            pv_a = alloc_pv(f"pvA{i}")
                        if jj == 4:
                            pv_b = alloc_pv(f"pvB{i}")
                        if jj < 4:
                            emit_pv_piece(i - 1, pv_a, 0, jj)
                        else:
                            emit_pv_piece(i - 1, pv_b, 1, jj - 4)
                        if jj == 3:
                            emit_norm_half(i - 1, 0, pv_a)
                if i == 0:
                    for kblk in range(8, NKB):
                        emit_v_proj(kblk)
                if pv_b is not None:
                    emit_norm_half(i - 1, 1, pv_b)
                    del pT_tiles[i - 1]
                    del pT2_tiles[i - 1]

        # ---------------- tail ----------------
        # st closes now; transposes + qh0 out-projection overlap PV(7).
        st_cm.__exit__(None, None, None)
        aT_cm = tc.tile_pool(name="ps_aT", bufs=2, space="PSUM")
        opj_cm = tc.tile_pool(name="ps_opj", bufs=4, space="PSUM")
        aT_pool[0] = aT_cm.__enter__()
        opj_pool[0] = opj_cm.__enter__()
        for j in range(4):
            emit_transpose(j)
        for qch in range(4):
            emit_opj(qch)
        emit_transpose(4)
        for qch in range(4, 8):
            emit_opj(qch)
        emit_transpose(5)
        pv_a = alloc_pv("pvA8")
        for jj in range(4):
            emit_pv_piece(7, pv_a, 0, jj)
        emit_transpose(6)
        emit_norm_half(7, 0, pv_a)
        pv_b = alloc_pv("pvB8")
        for jj in range(4):
            emit_pv_piece(7, pv_b, 1, jj)
        emit_norm_half(7, 1, pv_b)
        del pT_tiles[7]
        del pT2_tiles[7]
        emit_transpose(7)
        for qch in range(8, 16):
            emit_opj(qch)
        opj_cm.__exit__(None, None, None)
        aT_cm.__exit__(None, None, None)
        aux_cm.__exit__(None, None, None)


_CACHED_NC = None


def _get_nc():
    global _CACHED_NC
    if _CACHED_NC is not None:
        return _CACHED_NC
    nc = bacc.Bacc("TRN2", target_bir_lowering=False, debug=False)
    xqT = nc.dram_tensor("xqT", [D, S], F16, kind="ExternalInput").ap()
    xkT = nc.dram_tensor("xkT", [D, S], F16, kind="ExternalInput").ap()
    xvT = nc.dram_tensor("xvT", [D, S], F16, kind="ExternalInput").ap()
    wq = nc.dram_tensor("wq", [D, DG], F16, kind="ExternalInput").ap()
    wk = nc.dram_tensor("wk", [D, DG], F16, kind="ExternalInput").ap()
    wv = nc.dram_tensor("wv", [D, DG], F16, kind="ExternalInput").ap()
    wo = nc.dram_tensor("wo", [DG, D], F16, kind="ExternalInput").ap()
    ident = nc.dram_tensor("ident", [P, P], F16, kind="ExternalInput").ap()
    out = nc.dram_tensor("out", [S, D], F16, kind="ExternalOutput").ap()
    with tile.TileContext(nc) as tc:
        _build_mha(tc, out, xqT, xkT, xvT, wq, wk, wv, wo, ident)
    nc.compile()
    _CACHED_NC = nc
    return nc


def _run(in_query, in_key, in_value, W_q, W_k, W_v, W_out, **run_kwargs):
    f = lambda a: np.ascontiguousarray(np.asarray(a), dtype=np.float32)
    in_query, in_key, in_value = f(in_query), f(in_key), f(in_value)
    W_q, W_k, W_v, W_out = f(W_q), f(W_k), f(W_v), f(W_out)
    g16 = lambda a: np.ascontiguousarray(a, dtype=np.float16)
    xqT = [g16(in_query[b].T) for b in range(B)]
    xkT = [g16(in_key[b].T) for b in range(B)]
    xvT = [g16(in_value[b].T) for b in range(B)]
    ident = np.eye(P, dtype=np.float16)
    in_maps = []
    for c in range(NCORES):
        b, g = c // 2, c % 2
        in_maps.append(
            {
                "xqT": xqT[b],
                "xkT": xkT[b],
                "xvT": xvT[b],
                "wq": g16(W_q[:, g * DG : (g + 1) * DG]),
                "wk": g16(W_k[:, g * DG : (g + 1) * DG]),
                "wv": g16(W_v[:, g * DG : (g + 1) * DG]),
                "wo": np.ascontiguousarray(
                    W_out[g * DG : (g + 1) * DG, :].astype(np.float16)
                ),
                "ident": ident,
            }
        )
    res = run_bass_kernel_spmd(_get_nc(), in_maps, list(range(NCORES)), **run_kwargs)
    out = np.empty((B, S, D), np.float32)
    for b in range(B):
        out[b] = res.results[2 * b]["out"].astype(np.float32) + res.results[
            2 * b + 1
        ]["out"].astype(np.float32)
    return out, res


def kernel(in_query, in_key, in_value, W_q, W_k, W_v, W_out):
    out, _ = _run(in_query, in_key, in_value, W_q, W_k, W_v, W_out)
    return out


# revision 32
# speedup vs baseline: 1.0112x; 1.0112x over previous
"""Multi-head attention (B=4, S=2048, D=512, H=8) on 8 TRN2 NeuronCores.

Sharding: core c handles batch b = c//2 and head-group g = c%2 (heads
4g..4g+3, a 256-dim slice of the model). Attention is independent per
(batch, head); the out-projection contracts over all 512 dims, so each core
computes a partial out = attn_g @ W_out[g] and the host sums the two
partials per batch. x^T inputs are marshalled on the host; projection
weights are sliced per head-group (no K/V duplication).

Per-core dataflow:
  1. qT/kT = (x W)^T in [256, 2048] dims-major fp32r layout; v in natural
     [2048, 256] + a ones column per (key-block, head) -> v_aug fp16.
  2. Per slot (query-half, head): scores^T st[128k, 1024q] fp32r in PSUM;
     softmax numerators pT[128k, 1024q] fp16 produced by a mixed engine
     schedule: ACT exact exp / DVE+Pool Schraudolph int16-bitcast exp
     (single or phase-averaged pair for 4x better accuracy). Pool cannot
     read PSUM, so its inputs are DMA-staged to SBUF.
  3. PV in natural orientation: pv[128q, 65] += pT^T @ v_aug per
     (q-block, key-block) - the 65-wide free dim (64 dims + denominator
     column) makes each accumulation step cost 65 PE cycles instead of
     the 1024 a dims-major PV would pay. Normalization (1/denominator)
     is fused into the PSUM->SBUF copy via a per-partition scalar.
  4. attn[q,dh] tiles are transposed back to dims-major via PE-transpose
     against a host-provided identity, then out = attnT^T @ W_out (fp16)
     streams per 128-query chunk.
"""

import numpy as np

import concourse.bass as bass
from concourse import bacc
import concourse.mybir as mybir
import concourse.tile as tile
from concourse.bass_utils import run_bass_kernel_spmd

B, S, D, H = 4, 2048, 512, 8
DH = 64
P = 128
NCORES = 8
HG = H // 2          # 4 heads per core
DG = HG * DH         # 256 dims per core
NKB = S // P         # 16 key blocks
VW = DH + 1          # 65
F32 = mybir.dt.float32
F32R = mybir.dt.float32r
F16 = mybir.dt.float16
I16 = mybir.dt.int16
EXP = mybir.ActivationFunctionType.Exp
MUL = mybir.AluOpType.mult
ADD = mybir.AluOpType.add
SCALE = 1.0 / np.sqrt(DH)            # 0.125
LOG2E = 1.4426950408889634
A16 = SCALE * LOG2E * 1024.0         # fp16-bits Schraudolph slope
B_SS = 15.0 * 1024.0 - 59.0          # single Schraudolph bias (mean-1)
# Equal-weight phase pair: exp(x) ~ bits(x*A+B_PH1).f16 + bits(x*A+B_PH2).f16
# (CV 0.53%, zero mean bias; the 0.5 weights are folded into the biases so
# the combine is a single fp16 add, which Pool supports.)
B_PH1 = 15.0 * 1024.0 - 1024.0 - 332.0
B_OFF = 504.0  # S2 = S1 + B_OFF in int16 bits (half-period phase shift)

# Per-slot exp engine schedule over the 16 key blocks:
#  A  = exact exp on ACT
#  PH = phase-averaged Schraudolph pair on DVE; the two halves are never
#       added explicitly - PV accumulates both (PV is linear in p), so the
#       fp16 "combine" costs a few extra 65-cycle PE matmuls instead of a
#       slow Pool tensor op.
SCHED = ["A", "PH", "A", "D", "A", "A", "PH", "A",
         "A", "D", "A", "D", "A", "PH", "A", "A"]
PH_BLOCKS = [k for k, s in enumerate(SCHED) if s == "PH"]
PH_IDX = {k: i for i, k in enumerate(PH_BLOCKS)}


def _build_mha(tc, out_d, xqT_d, xkT_d, xvT_d, wq_d, wk_d, wv_d, wo_d, ident_d):
    nc = tc.nc

    dma_rr = [0]

    def dma(dst, src):
        eng = nc.sync if dma_rr[0] % 2 == 0 else nc.scalar
        dma_rr[0] += 1
        eng.dma_start(dst, src)

    copy_rr = [0]

    def pcopy(dst, src):
        if copy_rr[0] % 2 == 0:
            nc.scalar.copy(dst, src)
        else:
            nc.vector.tensor_copy(dst, src)
        copy_rr[0] += 1

    with (
        tc.tile_pool(name="consts", bufs=1) as cpool,
        tc.tile_pool(name="big", bufs=1) as bpool,
        tc.tile_pool(name="work", bufs=2) as wpool,
    ):
        wq_sb = cpool.tile([P, 4, DG], F16)
        wk_sb = cpool.tile([P, 4, DG], F16)
        wv_sb = cpool.tile([P, 4, DG], F16)
        wo_sb = cpool.tile([P, 2, D], F16)
        ident = cpool.tile([P, P], F16)
        xq_sb = bpool.tile([P, 4, S], F16)   # [din%128, din//128, s]
        xk_sb = bpool.tile([P, 4, S], F16)
        xv_sb = bpool.tile([P, 4, S], F16)
        qT = bpool.tile([P, 2, S], F16)      # [dim%128, dim//128, q]
        kT = bpool.tile([P, 2, S], F16)
        v_aug = bpool.tile([P, NKB, HG, VW], F16)  # [k%128, k//128, h, dh|1]
        attnT = bpool.tile([P, 2, S], F16)   # [dim%128, dim//128, q]
        scratch = cpool.tile([P, 512], F16)
        warm = cpool.tile([P, 1], F16)

        # gpsimd lane: scratch memset + ones-fill first, then SWDGE DMAs
        # (which bypass the serialized HWDGE transfer resource).
        nc.gpsimd.memset(scratch, 0)
        nc.gpsimd.tensor_scalar(
            out=v_aug.rearrange("p a b e -> p (a b) e")[:, :, DH],
            in0=scratch[:, 0:1].broadcast_to([P, NKB * HG]),
            scalar1=0.0,
            scalar2=1.0,
            op0=MUL,
            op1=ADD,
        )

        # Pull the ACT exp-table load to t=0 (input values are arbitrary).
        nc.scalar.activation(warm, scratch[:, 0:1], EXP)

        # Transfers share one serialized DMA resource served round-robin
        # across queue heads: half-strips, arrival-ordered by need (q first,
        # then k, then v/wo/ident).
        HS = S // 2
        nc.sync.dma_start(wq_sb, wq_d.rearrange("(c p) n -> p c n", p=P))
        nc.scalar.dma_start(wk_sb, wk_d.rearrange("(c p) n -> p c n", p=P))
        for h in range(2):
            sl = slice(h * HS, (h + 1) * HS)
            for c in range(2):
                nc.sync.dma_start(xq_sb[:, c, sl], xqT_d[c * P : (c + 1) * P, sl])
                nc.gpsimd.dma_start(
                    xq_sb[:, 2 + c, sl], xqT_d[(2 + c) * P : (3 + c) * P, sl]
                )
        for h in range(2):
            sl = slice(h * HS, (h + 1) * HS)
            for c in range(2):
                nc.gpsimd.dma_start(xk_sb[:, c, sl], xkT_d[c * P : (c + 1) * P, sl])
                nc.scalar.dma_start(
                    xk_sb[:, 2 + c, sl], xkT_d[(2 + c) * P : (3 + c) * P, sl]
                )
        nc.scalar.dma_start(wv_sb, wv_d.rearrange("(c p) n -> p c n", p=P))
        for c in range(4):
            nc.gpsimd.dma_start(xv_sb[:, c, :], xvT_d[c * P : (c + 1) * P, :])
        nc.scalar.dma_start(wo_sb, wo_d.rearrange("(c p) n -> p c n", p=P))
        nc.sync.dma_start(ident, ident_d)

        # ---------------- attention pools (aux outermost, then st) --------
        # aux carries projection psums, then PV psums, then the final PV(7);
        # one uniform [P, 512] tag so its 2 banks are reused across phases.
        aux_cm = tc.tile_pool(name="ps_aux", bufs=2, space="PSUM")
        aux = aux_cm.__enter__()
        st_cm = tc.tile_pool(name="ps_st", bufs=3, space="PSUM")
        st_pool = st_cm.__enter__()

        # PE pre-ramp into a throwaway st-pool tile.
        ramp = st_pool.tile([P, 1024], F32, tag="st", name="ramp")
        for _ in range(13):
            nc.tensor.matmul(
                ramp[:, 0:512], scratch[:, 0:P], scratch, start=True, stop=True
            )

        def project_col(x_sb, w_sb, dst, mc, col):
            pp = aux.tile([P, 512], F32, tag="qk", name=f"pp{mc}{col}")
            for c in range(4):
                nc.tensor.matmul(
                    pp,
                    w_sb[:, c, mc * P : (mc + 1) * P],
                    x_sb[:, c, col * 512 : (col + 1) * 512],
                    start=(c == 0),
                    stop=(c == 3),
                )
            pcopy(dst[:, mc, col * 512 : (col + 1) * 512], pp)

        # Head: only the mc0 halves of Q/K (all slot-0/1 scores need just
        # dims 0..127); mc1 halves + V weave into slots 0/1 as PE filler.
        for col in range(4):
            project_col(xq_sb, wq_sb, qT, 0, col)
        for col in range(4):
            project_col(xk_sb, wk_sb, kT, 0, col)

        def emit_v_proj(kblk):
            # V projection psum shares the aux qk buffers (1 bank each).
            vt = aux.tile([P, 512], F32, tag="qk", name=f"vp{kblk}")
            vv = vt[:, 0:260].rearrange("p (a b) -> p a b", b=VW)
            for c in range(4):
                nc.tensor.matmul(
                    vv[:, :, 0:DH],
                    xv_sb[:, c, kblk * P : (kblk + 1) * P],
                    wv_sb[:, c, :],
                    start=(c == 0),
                    stop=(c == 3),
                )
            pcopy(v_aug[:, kblk, :, 0:DH], vv[:, :, 0:DH])

        # ---------------- attention slots ----------------
        slots = [(qh, h) for qh in (0, 1) for h in range(4)]

        def alloc_pv(name):
            t = aux.tile([P, 512], F32, tag="qk", name=name)
            return t[:, 0:260].rearrange("p (a b) -> p a b", b=VW)
        aT_pool = [None]
        opj_pool = [None]

        pT_tiles = {}
        pT2_tiles = {}
        attn_tiles = {}

        def emit_score_block(i, kblk):
            qh, h = slots[i]
            po, mc = DH * (h % 2), h // 2
            pT = pT_tiles[i]
            st = st_pool.tile([P, 1024], F32, tag="st")
            qoff = qh * 1024
            for qc in range(2):
                nc.tensor.matmul(
                    st[:, qc * 512 : (qc + 1) * 512],
                    kT[po : po + DH, mc, kblk * P : (kblk + 1) * P],
                    qT[po : po + DH, mc, qoff + qc * 512 : qoff + (qc + 1) * 512],
                    start=True,
                    stop=True,
                )
            kind = SCHED[kblk]
            dst16 = pT[:, kblk, :]
            if kind == "A":
                nc.scalar.activation(dst16, st, EXP, scale=float(SCALE))
            elif kind == "D":
                nc.vector.tensor_scalar(
                    out=dst16.bitcast(I16), in0=st,
                    scalar1=A16, scalar2=B_SS, op0=MUL, op1=ADD,
                )
            else:  # PH: half 1 into pT, half 2 (bits + B_OFF) into pT2
                nc.vector.tensor_scalar(
                    out=dst16.bitcast(I16), in0=st,
                    scalar1=A16, scalar2=B_PH1, op0=MUL, op1=ADD,
                )
                pT2 = pT2_tiles[i]
                nc.gpsimd.tensor_scalar(
                    out=pT2.bitcast(I16)[:, PH_IDX[kblk], :],
                    in0=dst16.bitcast(I16),
                    scalar1=float(B_OFF),
                    scalar2=None,
                    op0=ADD,
                )

        def emit_pv_piece(j, pv, half, jj):
            qh, h = slots[j]
            pT = pT_tiles[j]
            pT2 = pT2_tiles[j]
            qblk = half * 4 + jj
            for kblk in range(NKB):
                nc.tensor.matmul(
                    pv[:, jj, :],
                    pT[:, kblk, qblk * P : (qblk + 1) * P],
                    v_aug[:, kblk, h, :],
                    start=(kblk == 0),
                    stop=False,
                )
            for pi, kblk in enumerate(PH_BLOCKS):
                nc.tensor.matmul(
                    pv[:, jj, :],
                    pT2[:, pi, qblk * P : (qblk + 1) * P],
                    v_aug[:, kblk, h, :],
                    start=False,
                    stop=(pi == len(PH_BLOCKS) - 1),
                )

        def emit_norm_half(j, half, pv):
            if half == 0:
                attn_tiles[j] = wpool.tile([P, 8, DH], F16, tag="attn", bufs=8, name=f"attn{j}")
            attn = attn_tiles[j]
            recip = wpool.tile([P, 4], F32, tag="recip", bufs=2)
            nc.vector.reciprocal(recip, pv[:, :, DH])
            for jj in range(4):
                nc.vector.tensor_scalar(
                    out=attn[:, half * 4 + jj, :],
                    in0=pv[:, jj, 0:DH],
                    scalar1=recip[:, jj : jj + 1],
                    scalar2=None,
                    op0=MUL,
                )

        def emit_transpose(j):
            qh, h = slots[j]
            po, mc = DH * (h % 2), h // 2
            attn = attn_tiles.pop(j)
            aT = aT_pool[0].tile([DH, 1024], F16, tag="aT")
            for qblk in range(8):
                nc.tensor.transpose(
                    aT[:, qblk * P : (qblk + 1) * P], attn[:, qblk, :], ident
                )
            nc.vector.tensor_copy(
                attnT[po : po + DH, mc, qh * 1024 : (qh + 1) * 1024], aT
            )

        def emit_opj(qch):
            op = opj_pool[0].tile([P, D], F32, tag="opj")
            for c in range(2):
                nc.tensor.matmul(
                    op,
                    attnT[:, c, qch * P : (qch + 1) * P],
                    wo_sb[:, c, :],
                    start=(c == 0),
                    stop=(c == 1),
                )
            ob = wpool.tile([P, D], F16, tag="ob", bufs=4)
            if qch % 2 == 0:
                nc.scalar.copy(ob, op)
            else:
                nc.vector.tensor_copy(ob, op)
            oq = (nc.gpsimd, nc.sync, nc.gpsimd, nc.scalar)[qch % 4]
            oq.dma_start(out_d[qch * P : (qch + 1) * P, :], ob)

        for i in range(8):
            pv_a = pv_b = None
            if True:
                pT_tiles[i] = wpool.tile([P, NKB, 1024], F16, tag="pT", bufs=2, name=f"pT{i}")
                pT2_tiles[i] = wpool.tile(
                    [P, len(PH_BLOCKS), 1024], F16, tag="pT2", bufs=2, name=f"pT2_{i}"
                )
                for kblk in range(NKB):
                    emit_score_block(i, kblk)
                    if i == 0:
                        if kblk < 4:
                            project_col(xq_sb, wq_sb, qT, 1, kblk)
                        elif kblk < 8:
                            project_col(xk_sb, wk_sb, kT, 1, kblk - 4)
                        else:
                            emit_v_proj(kblk - 8)
                    if i >= 1 and kblk % 2 == 1:
                        # weave one PV(i-1) q-block between score pairs
                        jj = (kblk - 1) // 2
                        if jj == 0:
                            pv_a = alloc_pv(f"pvA{i}")
                        if jj == 4:
                            pv_b = alloc_pv(f"pvB{i}")
                        if jj < 4:
                            emit_pv_piece(i - 1, pv_a, 0, jj)
                        else:
                            emit_pv_piece(i - 1, pv_b, 1, jj - 4)
                        if jj == 3:
                            emit_norm_half(i - 1, 0, pv_a)
                if i == 0:
                    for kblk in range(8, NKB):
                        emit_v_proj(kblk)
                if pv_b is not None:
                    emit_norm_half(i - 1, 1, pv_b)
                    del pT_tiles[i - 1]
                    del pT2_tiles[i - 1]

        # ---------------- tail ----------------
        # st closes now; transposes + qh0 out-projection overlap PV(7).
        st_cm.__exit__(None, None, None)
        aT_cm = tc.tile_pool(name="ps_aT", bufs=2, space="PSUM")
        opj_cm = tc.tile_pool(name="ps_opj", bufs=4, space="PSUM")
        aT_pool[0] = aT_cm.__enter__()
        opj_pool[0] = opj_cm.__enter__()
        for j in range(4):
            emit_transpose(j)
        for qch in range(4):
            emit_opj(qch)
        emit_transpose(4)
        for qch in range(4, 8):
            emit_opj(qch)
        emit_transpose(5)
        pv_a = alloc_pv("pvA8")
        for jj in range(4):
            emit_pv_piece(7, pv_a, 0, jj)
        emit_transpose(6)
        emit_norm_half(7, 0, pv_a)
        pv_b = alloc_pv("pvB8")
        for jj in range(4):
            emit_pv_piece(7, pv_b, 1, jj)
        emit_norm_half(7, 1, pv_b)
        del pT_tiles[7]
        del pT2_tiles[7]
        emit_transpose(7)
        for qch in range(8, 16):
            emit_opj(qch)
        opj_cm.__exit__(None, None, None)
        aT_cm.__exit__(None, None, None)
        aux_cm.__exit__(None, None, None)


_CACHED_NC = None


def _get_nc():
    global _CACHED_NC
    if _CACHED_NC is not None:
        return _CACHED_NC
    nc = bacc.Bacc("TRN2", target_bir_lowering=False, debug=False)
    xqT = nc.dram_tensor("xqT", [D, S], F16, kind="ExternalInput").ap()
    xkT = nc.dram_tensor("xkT", [D, S], F16, kind="ExternalInput").ap()
    xvT = nc.dram_tensor("xvT", [D, S], F16, kind="ExternalInput").ap()
    wq = nc.dram_tensor("wq", [D, DG], F16, kind="ExternalInput").ap()
    wk = nc.dram_tensor("wk", [D, DG], F16, kind="ExternalInput").ap()
    wv = nc.dram_tensor("wv", [D, DG], F16, kind="ExternalInput").ap()
    wo = nc.dram_tensor("wo", [DG, D], F16, kind="ExternalInput").ap()
    ident = nc.dram_tensor("ident", [P, P], F16, kind="ExternalInput").ap()
    out = nc.dram_tensor("out", [S, D], F16, kind="ExternalOutput").ap()
    with tile.TileContext(nc) as tc:
        _build_mha(tc, out, xqT, xkT, xvT, wq, wk, wv, wo, ident)
    nc.compile()
    _CACHED_NC = nc
    return nc


def _run(in_query, in_key, in_value, W_q, W_k, W_v, W_out, **run_kwargs):
    f = lambda a: np.ascontiguousarray(np.asarray(a), dtype=np.float32)
    in_query, in_key, in_value = f(in_query), f(in_key), f(in_value)
    W_q, W_k, W_v, W_out = f(W_q), f(W_k), f(W_v), f(W_out)
    g16 = lambda a: np.ascontiguousarray(a, dtype=np.float16)
    xqT = [g16(in_query[b].T) for b in range(B)]
    xkT = [g16(in_key[b].T) for b in range(B)]
    xvT = [g16(in_value[b].T) for b in range(B)]
    ident = np.eye(P, dtype=np.float16)
    in_maps = []
    for c in range(NCORES):
        b, g = c // 2, c % 2
        in_maps.append(
            {
                "xqT": xqT[b],
                "xkT": xkT[b],
                "xvT": xvT[b],
                "wq": g16(W_q[:, g * DG : (g + 1) * DG]),
                "wk": g16(W_k[:, g * DG : (g + 1) * DG]),
                "wv": g16(W_v[:, g * DG : (g + 1) * DG]),
                "wo": np.ascontiguousarray(
                    W_out[g * DG : (g + 1) * DG, :].astype(np.float16)
                ),
                "ident": ident,
            }
        )
    res = run_bass_kernel_spmd(_get_nc(), in_maps, list(range(NCORES)), **run_kwargs)
    out = np.empty((B, S, D), np.float32)
    for b in range(B):
        out[b] = res.results[2 * b]["out"].astype(np.float32) + res.results[
            2 * b + 1
        ]["out"].astype(np.float32)
    return out, res


def kernel(in_query, in_key, in_value, W_q, W_k, W_v, W_out):
    out, _ = _run(in_query, in_key, in_value, W_q, W_k, W_v, W_out)
    return out
